# revision 50
# baseline (speedup 1.0000x reference)
"""DiT block (windowed attention + AdaLN-Zero + MLP) on 8 Trainium2 cores.

Sharding: data-parallel over the 64 independent attention windows
(B=2 x 32 windows of 8x8x8 tokens). Each core processes 8 windows =
4096 tokens end-to-end (LN1+mod, qkv, windowed attention, proj,
residual, LN2+mod, MLP, residual) with zero collectives. The host
performs only layout work: window partitioning + transpose to
feature-major [C, tokens] per core (DRAM arrays are laid out exactly
like the SBUF tiles so every DMA is contiguous), and the inverse on
the way out.

Device layout is feature-major (channels on partitions, tokens free):
every matmul takes weights as lhsT (native [Cin, Cout] layout) and
activations as rhs, producing feature-major outputs directly. LN
statistics are computed with (1/C)-scaled ones-vector matmuls on the
PE; per-token stats are broadcast across partitions with K=1 matmuls.
The AdaLN shift (+LN beta) term is folded into per-output-feature
biases of the following matmul (qkv / mlp1), so normalize+modulate is
2 fused DVE ops per [128, 512] chunk.

Attention per head: S^T tiles [128 kt, 512 qt] via K=64 matmuls
(q pre-scaled by 1/8), exp on ACT (no max subtraction needed: scores
are O(1) by construction), then AV with a ones-augmented V stationary
operand so the softmax denominators fall out of the same matmuls
(psum row 64). Normalization 1/sums is broadcast to the right
(head, partition) slots with masked K=1 matmuls.

Matmuls run in float32r (full-rate fp32, TF32-ish rounding on HW);
the mlp2 contraction runs in bf16 (hidden activations + w2) to fit
SBUF, which only perturbs the gated FFN branch.
"""

import numpy as np

import concourse.bass as bass
import concourse.tile as tile
from concourse import bacc
from concourse import mybir
from concourse.bass_utils import run_bass_kernel_spmd

F32 = mybir.dt.float32
R32 = mybir.dt.float32r
BF16 = mybir.dt.bfloat16

B = 2
T, H, W = 16, 32, 32
N = T * H * W
C = 512
DFF = 2048
NH = 8
HD = 64
WIN = 8
NTOK_CORE = 4096   # 8 windows * 512 tokens
NWIN_CORE = 8
TOKW = 512         # tokens per window
KC = 4             # C / 128 chunks
EPS = 1e-6
NCORES = 8

_CACHE = {}


def _build_program(gelu_identity=False, reps=1):
    nc = bacc.Bacc(None, target_bir_lowering=False, debug=False)

    x_in = nc.declare_dram_parameter("x_in", [NWIN_CORE, 128, KC, TOKW], R32, isOutput=False)
    c_col = nc.declare_dram_parameter("c_col", [128, KC], F32, isOutput=False)
    qkv_w = nc.declare_dram_parameter("qkv_w", [128, KC, 3 * C], R32, isOutput=False)
    proj_w = nc.declare_dram_parameter("proj_w", [128, KC, C], R32, isOutput=False)
    projb_row = nc.declare_dram_parameter("projb_row", [1, C], R32, isOutput=False)
    mlp_w1 = nc.declare_dram_parameter("mlp_w1", [128, KC, DFF], R32, isOutput=False)
    mlp_w2 = nc.declare_dram_parameter("mlp_w2", [128, 16, C], F32, isOutput=False)
    ada_w = nc.declare_dram_parameter("ada_w", [24, 128, 512], R32, isOutput=False)
    ada_b_c = nc.declare_dram_parameter("ada_b_c", [128, 6 * KC], F32, isOutput=False)
    g1_col = nc.declare_dram_parameter("g1_col", [128, KC], F32, isOutput=False)
    b1_col = nc.declare_dram_parameter("b1_col", [128, KC], F32, isOutput=False)
    g2_col = nc.declare_dram_parameter("g2_col", [128, KC], F32, isOutput=False)
    b2_col = nc.declare_dram_parameter("b2_col", [128, KC], F32, isOutput=False)
    y_out = nc.declare_dram_parameter("y_out", [NWIN_CORE, 128, KC, TOKW], F32, isOutput=True)

    MUL = mybir.AluOpType.mult
    ADD = mybir.AluOpType.add
    ACT_F = mybir.ActivationFunctionType

    with tile.TileContext(nc) as tc:
        with (
            tc.tile_pool(name="sb", bufs=1) as sb,
            tc.tile_pool(name="ps", bufs=8, space="PSUM") as psp,
            tc.tile_pool(name="dr", bufs=2, space="DRAM") as drp,
        ):
            def pst(m=128):
                return psp.tile([m, 512], F32, tag="ps", name="ps_t")

            def row_to_cols(row, dst_cols, m):
                """Transpose a [1, m*128] SBUF row into [128, m] columns
                via a DRAM bounce (SBUF->SBUF DMA can't balance these APs)."""
                d = drp.tile([m * 128], R32, tag="dr", name="dr_t")
                nc.sync.dma_start(out=d[:], in_=row[0:1, 0 : m * 128])
                nc.sync.dma_start(
                    out=dst_cols,
                    in_=d.rearrange("(m p) -> p m", p=128).bitcast(dst_cols.dtype),
                )

            # ---------------- constants / once-per-core ----------------
            ones_row = sb.tile([1, TOKW], R32, tag="ones_row")
            nc.vector.memset(ones_row.bitcast(F32), 1.0)
            invc_col = sb.tile([128, 1], R32, tag="invc_col")
            nc.vector.memset(invc_col.bitcast(F32), 1.0 / C)
            # [1,128] masks selecting the lower/upper 64 partitions (used to
            # broadcast per-head softmax denominators to the right half).
            sel_lo = sb.tile([1, 128], R32, tag="sel_lo")
            sel_hi = sb.tile([1, 128], R32, tag="sel_hi")
            nc.vector.memset(sel_lo.bitcast(F32)[:, 0:64], 1.0)
            nc.vector.memset(sel_lo.bitcast(F32)[:, 64:128], 0.0)
            nc.vector.memset(sel_hi.bitcast(F32)[:, 0:64], 0.0)
            nc.vector.memset(sel_hi.bitcast(F32)[:, 64:128], 1.0)

            eps_t = sb.tile([1, 1], F32, tag="row", bufs=2)
            nc.vector.memset(eps_t, EPS)

            c_sb = sb.tile([128, KC], F32, tag="c_sb")
            nc.sync.dma_start(out=c_sb, in_=c_col[:, :])
            sig_c = sb.tile([128, KC], F32, tag="sig_c")
            nc.scalar.activation(sig_c, c_sb, ACT_F.Sigmoid)
            silu_c = sb.tile([128, KC], R32, tag="silu_c")
            nc.vector.tensor_tensor(out=silu_c, in0=c_sb, in1=sig_c, op=MUL)

            ada_b_sb = sb.tile([128, 6 * KC], F32, tag="ada_b_sb")
            nc.sync.dma_start(out=ada_b_sb, in_=ada_b_c[:, :])

            # ---------------- big weights (contiguous DMAs) ----------------
            qkv_sb = sb.tile([128, KC, 3 * C], R32, tag="qkv_sb")
            nc.sync.dma_start(out=qkv_sb, in_=qkv_w[:, :, :])
            proj_sb = sb.tile([128, KC, C], R32, tag="proj_sb")
            nc.scalar.dma_start(out=proj_sb, in_=proj_w[:, :, :])
            w1_sb = sb.tile([128, KC, DFF], R32, tag="w1_sb")
            nc.scalar.dma_start(out=w1_sb, in_=mlp_w1[:, :, :])
            w2_sb = sb.tile([128, 16, C], BF16, tag="w2_sb")
            nc.gpsimd.dma_start(out=w2_sb, in_=mlp_w2[:, :, :])  # casts f32->bf16
            projb_sb = sb.tile([1, C], R32, tag="projb_sb")
            nc.sync.dma_start(out=projb_sb, in_=projb_row[:, :])

            # mod = silu(c) @ ada_w + ada_b via M=1 matmuls; ada_w streamed
            # through rotating contiguous tiles; results transposed into
            # per-partition columns modT.
            modT = sb.tile([128, 6 * KC], F32, tag="modT")
            for n in range(6):
                ps_m = psp.tile([1, 512], F32, tag="ps", name="ps_m")
                for k in range(KC):
                    aw = sb.tile([128, 512], R32, tag="exp_t", bufs=2, name="aw")
                    (nc.sync if k % 2 == 0 else nc.scalar).dma_start(out=aw, in_=ada_w[n * KC + k])
                    nc.tensor.matmul(
                        ps_m[0:1, :],
                        silu_c[:, k : k + 1],
                        aw,
                        start=(k == 0),
                        stop=(k == KC - 1),
                    )
                row = sb.tile([1, TOKW], R32, tag="row", bufs=2, name="row")
                nc.vector.tensor_copy(row, ps_m[0:1, :])
                row_to_cols(row, modT[:, n * KC : (n + 1) * KC], KC)
            nc.vector.tensor_tensor(out=modT, in0=modT, in1=ada_b_sb, op=ADD)

            # Per-channel affine folds. j: 0 shift_mha, 1 scale_mha,
            # 2 gate_mha, 3 shift_ffn, 4 scale_ffn, 5 gate_ffn.
            g1_sb = sb.tile([128, KC], F32, tag="g1_sb")
            b1_sb = sb.tile([128, KC], F32, tag="b1_sb")
            g2_sb = sb.tile([128, KC], F32, tag="g2_sb")
            b2_sb = sb.tile([128, KC], F32, tag="b2_sb")
            nc.sync.dma_start(out=g1_sb, in_=g1_col[:, :])
            nc.sync.dma_start(out=b1_sb, in_=b1_col[:, :])
            nc.sync.dma_start(out=g2_sb, in_=g2_col[:, :])
            nc.sync.dma_start(out=b2_sb, in_=b2_col[:, :])

            A1 = sb.tile([128, KC], F32, tag="A1")
            B1 = sb.tile([128, KC], R32, tag="B1")
            A2 = sb.tile([128, KC], F32, tag="A2")
            B2 = sb.tile([128, KC], R32, tag="B2")
            tmpc = sb.tile([128, KC], F32, tag="tmpc")
            # A1 = gamma1*(1+scale_mha); B1 = beta1*(1+scale_mha)+shift_mha
            nc.vector.tensor_scalar_add(tmpc, modT[:, 1 * KC : 2 * KC], 1.0)
            nc.vector.tensor_tensor(out=A1, in0=g1_sb, in1=tmpc, op=MUL)
            nc.vector.scalar_tensor_tensor(
                out=B1, in0=b1_sb, scalar=1.0, in1=tmpc, op0=MUL, op1=MUL
            )
            nc.vector.tensor_tensor(out=B1, in0=B1, in1=modT[:, 0 * KC : 1 * KC], op=ADD)
            nc.vector.tensor_scalar_add(tmpc, modT[:, 4 * KC : 5 * KC], 1.0)
            nc.vector.tensor_tensor(out=A2, in0=g2_sb, in1=tmpc, op=MUL)
            nc.vector.scalar_tensor_tensor(
                out=B2, in0=b2_sb, scalar=1.0, in1=tmpc, op0=MUL, op1=MUL
            )
            nc.vector.tensor_tensor(out=B2, in0=B2, in1=modT[:, 3 * KC : 4 * KC], op=ADD)
            GATE1 = modT[:, 2 * KC : 3 * KC]
            GATE2 = modT[:, 5 * KC : 6 * KC]

            # qkv bias fold: qb = B1^T @ qkv_w [1536]; chunks 0-1 (q, k)
            # land as per-partition columns, chunk 2 (v) is broadcast to a
            # [128, 512] tile since v is produced token-major.
            qkb_cols = sb.tile([128, 8], F32, tag="qkb_cols")
            vb_bc = sb.tile([128, C], F32, tag="vb_bc")
            for n in range(3):
                ps_qb = psp.tile([1, 512], F32, tag="ps", name="ps_qb")
                for k in range(KC):
                    nc.tensor.matmul(
                        ps_qb[0:1, :],
                        B1[:, k : k + 1],
                        qkv_sb[:, k, n * 512 : (n + 1) * 512],
                        start=(k == 0),
                        stop=(k == KC - 1),
                    )
                row = sb.tile([1, TOKW], R32, tag="row", bufs=2, name="row")
                nc.vector.tensor_copy(row, ps_qb[0:1, :])
                if n < 2:
                    row_to_cols(row, qkb_cols[:, n * KC : (n + 1) * KC], KC)
                else:
                    ps_vb = pst()
                    nc.tensor.matmul(
                        ps_vb, ones_row[0:1, 0:128], row, start=True, stop=True
                    )
                    nc.vector.tensor_copy(vb_bc, ps_vb)
            qb8 = sb.tile([128, KC], F32, tag="qb8")
            nc.vector.tensor_scalar_mul(qb8, qkb_cols[:, 0:KC], 0.125)

            # mlp1 bias fold: hb = B2^T @ mlp_w1 [2048] as columns [128, 16].
            hb_cols = sb.tile([128, 16], F32, tag="hb_cols")
            for n in range(4):
                ps_hb = psp.tile([1, 512], F32, tag="ps", name="ps_hb")
                for k in range(KC):
                    nc.tensor.matmul(
                        ps_hb[0:1, :],
                        B2[:, k : k + 1],
                        w1_sb[:, k, n * 512 : (n + 1) * 512],
                        start=(k == 0),
                        stop=(k == KC - 1),
                    )
                row = sb.tile([1, TOKW], R32, tag="row", bufs=2, name="row")
                nc.vector.tensor_copy(row, ps_hb[0:1, :])
                row_to_cols(row, hb_cols[:, n * KC : (n + 1) * KC], KC)

            # ---------------- per-window pipeline ----------------
            def ln_stats(xin):
                """(1/C)-scaled ones matmuls: ps_mu = mean, ps_msq = E[x^2]."""
                ps_mu = psp.tile([1, 512], F32, tag="ps", name="ps_mu")
                ps_msq = psp.tile([1, 512], F32, tag="ps", name="ps_msq")
                for k in range(KC):
                    nc.tensor.matmul(
                        ps_mu[0:1, :], invc_col, xin[:, k, :],
                        start=(k == 0), stop=(k == KC - 1),
                    )
                for k in range(KC):
                    sq_t = sb.tile([128, TOKW], R32, tag="sqt", bufs=1, name="sq_t")
                    if k % 2 == 0:
                        nc.vector.tensor_tensor(
                            out=sq_t, in0=xin[:, k, :], in1=xin[:, k, :], op=MUL
                        )
                    else:
                        nc.scalar.activation(sq_t, xin[:, k, :], ACT_F.Square)
                    nc.tensor.matmul(
                        ps_msq[0:1, :], invc_col, sq_t,
                        start=(k == 0), stop=(k == KC - 1),
                    )
                return ps_mu, ps_msq

            def ln_rows(ps_mu, ps_msq):
                """Rows chain: rstd and -mean*rstd from the stats psums."""
                q_r = sb.tile([1, TOKW], F32, tag="q_r", bufs=1, name="q_r")
                rstd_r = sb.tile([1, TOKW], R32, tag="rstd_r", bufs=2, name="rstd_r")
                nb_r = sb.tile([1, TOKW], R32, tag="nb_r", bufs=2, name="nb_r")
                nc.scalar.activation(q_r, ps_mu[0:1, :], ACT_F.Square)
                nc.vector.scalar_tensor_tensor(
                    out=q_r, in0=q_r, scalar=-1.0,
                    in1=ps_msq[0:1, :], op0=MUL, op1=ADD,
                )  # var = E[x^2] - mean^2
                nc.scalar.activation(q_r, q_r, ACT_F.Sqrt, bias=eps_t[0:1, 0:1])
                with nc.allow_low_precision(reason="f32r rstd for matmul bcast"):
                    nc.vector.reciprocal(rstd_r, q_r)
                nc.vector.scalar_tensor_tensor(
                    out=nb_r, in0=ps_mu[0:1, :], scalar=-1.0,
                    in1=rstd_r, op0=MUL, op1=MUL,
                )  # -mean*rstd
                return rstd_r, nb_r

            def ln_apply(xin, xmod, Acol, rows):
                """Partition-broadcast + fused normalize*A.
                xmod = (LN(xin) w/o shift) * A; shift folded downstream."""
                rstd_r, nb_r = rows
                ps_a = pst()
                nc.tensor.matmul(
                    ps_a, ones_row[0:1, 0:128], rstd_r, start=True, stop=True
                )
                ps_b = pst()
                nc.tensor.matmul(
                    ps_b, ones_row[0:1, 0:128], nb_r, start=True, stop=True
                )
                for k in range(KC):
                    # u = (x*A) * rstd_bc ; xmod = nb_bc*A + u
                    nc.vector.scalar_tensor_tensor(
                        out=xmod[:, k, :], in0=xin[:, k, :],
                        scalar=Acol[:, k : k + 1], in1=ps_a, op0=MUL, op1=MUL,
                    )
                    nc.vector.scalar_tensor_tensor(
                        out=xmod[:, k, :], in0=ps_b,
                        scalar=Acol[:, k : k + 1], in1=xmod[:, k, :], op0=MUL, op1=ADD,
                    )

            def stage_a1(w):
                """Load x window + LN1 stats matmuls."""
                xw = sb.tile([128, KC, TOKW], R32, tag="xw", bufs=2, name="xw")
                nc.gpsimd.dma_start(out=xw, in_=x_in[w])
                ps_mu, ps_msq = ln_stats(xw)
                return xw, ps_mu, ps_msq

            def stage_a2(st, rows=None):
                """LN1 rows chain + modulate."""
                xw, ps_mu, ps_msq = st
                if rows is None:
                    rows = ln_rows(ps_mu, ps_msq)
                xmod = sb.tile([128, KC, TOKW], R32, tag="xmod", bufs=2, name="xmod")
                ln_apply(xw, xmod, A1, rows)
                return xmod

            def stage_qkv(xmod):
                q_sb = sb.tile([128, KC, TOKW], R32, tag="q_sb", name="q_sb")
                k_sb = sb.tile([128, KC, TOKW], R32, tag="k_sb", name="k_sb")
                for m in range(8):
                    ps_qk = pst()
                    for k in range(KC):
                        nc.tensor.matmul(
                            ps_qk,
                            qkv_sb[:, k, m * 128 : (m + 1) * 128],
                            xmod[:, k, :],
                            start=(k == 0), stop=(k == KC - 1),
                        )
                    if m < 4:
                        nc.scalar.activation(
                            q_sb[:, m, :], ps_qk, ACT_F.Identity,
                            bias=qb8[:, m : m + 1], scale=0.125,
                        )
                    else:
                        nc.vector.tensor_scalar_add(
                            k_sb[:, m - 4, :], ps_qk, qkb_cols[:, m : m + 1]
                        )
                v_aug = sb.tile([128, KC, NH, HD + 1], R32, tag="v_aug", name="v_aug")
                nc.vector.memset(v_aug.bitcast(F32)[:, :, :, HD : HD + 1], 1.0)
                for t4 in range(KC):
                    ps_v = pst()
                    for k in range(KC):
                        nc.tensor.matmul(
                            ps_v,
                            xmod[:, k, t4 * 128 : (t4 + 1) * 128],
                            qkv_sb[:, k, 2 * C : 3 * C],
                            start=(k == 0), stop=(k == KC - 1),
                        )
                    nc.vector.tensor_tensor(
                        out=v_aug[:, t4, :, 0:HD],
                        in0=ps_v.rearrange("p (h d) -> p h d", h=NH),
                        in1=vb_bc.rearrange("p (h d) -> p h d", h=NH),
                        op=ADD,
                    )
                return q_sb, k_sb, v_aug

            def stage_attn(q_sb, k_sb, v_aug, per_head_filler=None):
                """Windowed attention, head pairs (base partitions 0/64).
                The two S matmuls of a pair are emitted back-to-back so the
                PE can run them concurrently in disjoint row groups.
                per_head_filler(h) emits independent PE work (prev window's
                mlp2) to cover the ACT-bound exp."""
                o_sb = sb.tile([128, KC, TOKW], R32, tag="o_sb", name="o_sb")
                for p in range(KC):
                    h0, h1 = 2 * p, 2 * p + 1
                    ps_o0 = psp.tile([HD + 1, 512], F32, tag="ps", name="ps_o0")
                    ps_o1 = psp.tile([HD + 1, 512], F32, tag="ps", name="ps_o1")
                    for kc in range(KC):
                        ps_st0 = pst()
                        ps_st1 = pst()
                        nc.tensor.matmul(
                            ps_st0,
                            k_sb[0:64, p, kc * 128 : (kc + 1) * 128],
                            q_sb[0:64, p, :],
                            start=True, stop=True,
                        )
                        nc.tensor.matmul(
                            ps_st1,
                            k_sb[64:128, p, kc * 128 : (kc + 1) * 128],
                            q_sb[64:128, p, :],
                            start=True, stop=True,
                        )
                        exp_t0 = sb.tile([128, TOKW], R32, tag="exp_t", bufs=2, name="exp_t0")
                        nc.scalar.activation(exp_t0, ps_st0, ACT_F.Exp)
                        exp_t1 = sb.tile([128, TOKW], R32, tag="exp_t", bufs=2, name="exp_t1")
                        nc.scalar.activation(exp_t1, ps_st1, ACT_F.Exp)
                        nc.tensor.matmul(
                            ps_o0,
                            v_aug[:, kc, h0, :],
                            exp_t0,
                            start=(kc == 0), stop=(kc == KC - 1),
                        )
                        nc.tensor.matmul(
                            ps_o1,
                            v_aug[:, kc, h1, :],
                            exp_t1,
                            start=(kc == 0), stop=(kc == KC - 1),
                        )
                        if per_head_filler is not None and kc % 2 == 1:
                            per_head_filler(2 * p + kc // 2)
                    recip0 = sb.tile([1, TOKW], R32, tag="recip", bufs=2, name="recip0")
                    recip1 = sb.tile([1, TOKW], R32, tag="recip", bufs=2, name="recip1")
                    with nc.allow_low_precision(reason="f32r softmax denom"):
                        nc.vector.reciprocal(recip0, ps_o0[HD : HD + 1, :])
                        nc.vector.reciprocal(recip1, ps_o1[HD : HD + 1, :])
                    nc.vector.tensor_copy(o_sb[0:64, p, :], ps_o0[0:HD, :])
                    nc.vector.tensor_copy(o_sb[64:128, p, :], ps_o1[0:HD, :])
                    ps_r = pst()
                    nc.tensor.matmul(ps_r, sel_lo, recip0, start=True, stop=False)
                    nc.tensor.matmul(ps_r, sel_hi, recip1, start=False, stop=True)
                    nc.vector.tensor_tensor(
                        out=o_sb[:, p, :], in0=o_sb[:, p, :], in1=ps_r, op=MUL
                    )
                return o_sb

            def stage_proj(xw, o_sb):
                """proj + gated residual, written into the output tile
                (doubles as the residual stream for LN2/MLP/final add)."""
                out_w = sb.tile([128, KC, TOKW], R32, tag="out_sb", bufs=2, name="out_w")
                for m in range(KC):
                    ps_p = pst()
                    for kc in range(KC):
                        nc.tensor.matmul(
                            ps_p,
                            proj_sb[:, kc, m * 128 : (m + 1) * 128],
                            o_sb[:, kc, :],
                            start=(kc == 0), stop=False,
                        )
                    nc.tensor.matmul(
                        ps_p,
                        projb_sb[0:1, m * 128 : (m + 1) * 128],
                        ones_row,
                        start=False, stop=True,
                    )
                    nc.vector.scalar_tensor_tensor(
                        out=out_w[:, m, :], in0=ps_p,
                        scalar=GATE1[:, m : m + 1], in1=xw[:, m, :], op0=MUL, op1=ADD,
                    )
                return out_w

            def stage_mlp1(xmod2):
                hT = sb.tile([128, 16, TOKW], BF16, tag="hT", bufs=1, name="hT")
                for m in range(16):
                    ps_h = pst()
                    for kc in range(KC):
                        nc.tensor.matmul(
                            ps_h,
                            w1_sb[:, kc, m * 128 : (m + 1) * 128],
                            xmod2[:, kc, :],
                            start=(kc == 0), stop=(kc == KC - 1),
                        )
                    nc.scalar.activation(
                        hT[:, m, :], ps_h,
                        ACT_F.Identity if gelu_identity else ACT_F.Gelu,
                        bias=hb_cols[:, m : m + 1],
                    )
                return hT

            def mlp2_chunks(ps_y, hT, ki_list):
                for ki in ki_list:
                    for m4 in range(KC):
                        nc.tensor.matmul(
                            ps_y[m4],
                            w2_sb[:, ki, m4 * 128 : (m4 + 1) * 128],
                            hT[:, ki, :],
                            start=(ki == 0), stop=(ki == 15),
                        )

            def stage_out(w, out_w, ps_y):
                for m4 in range(KC):
                    nc.vector.scalar_tensor_tensor(
                        out=out_w[:, m4, :], in0=ps_y[m4],
                        scalar=GATE2[:, m4 : m4 + 1], in1=out_w[:, m4, :],
                        op0=MUL, op1=ADD,
                    )
                nc.gpsimd.dma_start(out=y_out[w], in_=out_w)

            # Software pipeline (engines execute their streams in order):
            #   attention(w) is interleaved with mlp2(w-1) so the
            #   ACT-bound exp phase has PE filler; LN chains of w/w+1 are
            #   covered by stats matmuls and mlp1; qkv(w+1) is emitted
            #   before attention(w+1) so window boundaries don't stall.
            wins = [wi for _ in range(reps) for wi in range(NWIN_CORE)]
            a1 = stage_a1(wins[0])
            xw_cur = a1[0]
            xmod_cur = stage_a2(a1)
            qkv_cur = stage_qkv(xmod_cur)
            prev = None  # (w, out_w, hT, ps_y)
            for idx, w in enumerate(wins):
                if prev is not None:
                    pw, pout, phT, pps_y = prev

                    def filler(h, _hT=phT, _ps_y=pps_y):
                        mlp2_chunks(_ps_y, _hT, [2 * h, 2 * h + 1])
                else:
                    filler = None
                o_sb = stage_attn(*qkv_cur, per_head_filler=filler)
                if prev is not None:
                    stage_out(pw, pout, pps_y)
                out_w = stage_proj(xw_cur, o_sb)
                ps_mu2, ps_msq2 = ln_stats(out_w)
                a1n = None
                if idx + 1 < len(wins):
                    a1n = stage_a1(wins[idx + 1])
                rows2 = ln_rows(ps_mu2, ps_msq2)
                rows1n = ln_rows(a1n[1], a1n[2]) if a1n is not None else None
                xmod2 = sb.tile([128, KC, TOKW], R32, tag="xmod", bufs=2, name="xmod2")
                ln_apply(out_w, xmod2, A2, rows2)
                hT = stage_mlp1(xmod2)
                ps_y = [pst() for _ in range(KC)]
                if a1n is not None:
                    xmod_nxt = stage_a2(a1n, rows=rows1n)
                    qkv_nxt = stage_qkv(xmod_nxt)
                prev = (w, out_w, hT, ps_y)
                if a1n is not None:
                    xw_cur = a1n[0]
                    xmod_cur = xmod_nxt
                    qkv_cur = qkv_nxt
            # epilogue: last window's mlp2 + store
            pw, pout, phT, pps_y = prev
            mlp2_chunks(pps_y, phT, list(range(16)))
            stage_out(pw, pout, pps_y)

    nc.compile()
    return nc


def _get_program():
    if "nc" not in _CACHE:
        _CACHE["nc"] = _build_program()
    return _CACHE["nc"]


def _window_partition(x):
    """[B, N, C] -> [64, 512, C] in reference window order."""
    xw = x.reshape(B, T // WIN, WIN, H // WIN, WIN, W // WIN, WIN, C)
    xw = xw.transpose(0, 1, 3, 5, 2, 4, 6, 7).reshape(-1, WIN**3, C)
    return xw


def _window_unpartition(ow):
    """[64, 512, C] -> [B, N, C]."""
    o = ow.reshape(B, T // WIN, H // WIN, W // WIN, WIN, WIN, WIN, C)
    o = o.transpose(0, 1, 4, 2, 5, 3, 6, 7).reshape(B, N, C)
    return o


def _col4(v):
    return np.ascontiguousarray(np.asarray(v, np.float32).reshape(KC, 128).T)


def _feat_major(wmat, kchunks):
    """[C_in, C_out] -> [128, kchunks, C_out] (SBUF tile layout)."""
    w = np.asarray(wmat, np.float32)
    return np.ascontiguousarray(
        w.reshape(kchunks, 128, w.shape[1]).transpose(1, 0, 2)
    )


def _make_in_maps(inputs):
    x = np.asarray(inputs["x"], np.float32)
    c = np.asarray(inputs["c"], np.float32)
    qkv_w = _feat_major(inputs["qkv_w"], KC)
    proj_w = _feat_major(inputs["proj_w"], KC)
    proj_b = np.ascontiguousarray(np.asarray(inputs["proj_b"], np.float32).reshape(1, C))
    mlp_w1 = _feat_major(inputs["mlp_w1"], KC)
    mlp_w2 = _feat_major(inputs["mlp_w2"], 16)
    aw = np.asarray(inputs["ada_w"], np.float32)
    # [24, 128, 512]: chunk (n, k) = rows k*128..(k+1)*128, cols n*512..(n+1)*512
    ada_w = np.ascontiguousarray(
        aw.reshape(KC, 128, 6, 512).transpose(2, 0, 1, 3).reshape(24, 128, 512)
    )
    ada_b = np.ascontiguousarray(
        np.asarray(inputs["ada_b"], np.float32)
        .reshape(6, KC, 128)
        .transpose(2, 0, 1)
        .reshape(128, 6 * KC)
    )
    g1 = _col4(inputs["gamma1"])
    b1 = _col4(inputs["beta1"])
    g2 = _col4(inputs["gamma2"])
    b2 = _col4(inputs["beta2"])

    xw = _window_partition(x)  # [64, 512, C]
    in_maps = []
    for i in range(NCORES):
        xi = xw[i * NWIN_CORE : (i + 1) * NWIN_CORE]  # [8, 512, C]
        # -> [8, 128, KC, 512]: x_in[w, p, k, t] = xi[w, t, k*128+p]
        x_in = np.ascontiguousarray(
            xi.reshape(NWIN_CORE, TOKW, KC, 128).transpose(0, 3, 2, 1)
        )
        in_maps.append(
            {
                "x_in": x_in,
                "c_col": _col4(c[i // (NCORES // B)]),
                "qkv_w": qkv_w,
                "proj_w": proj_w,
                "projb_row": proj_b,
                "mlp_w1": mlp_w1,
                "mlp_w2": mlp_w2,
                "ada_w": ada_w,
                "ada_b_c": ada_b,
                "g1_col": g1,
                "b1_col": b1,
                "g2_col": g2,
                "b2_col": b2,
            }
        )
    return in_maps


def _assemble(results):
    ow = np.empty((NCORES * NWIN_CORE, WIN**3, C), np.float32)
    for i in range(NCORES):
        yt = np.asarray(results[i]["y_out"])  # [8, 128, KC, 512]
        ow[i * NWIN_CORE : (i + 1) * NWIN_CORE] = yt.transpose(0, 3, 2, 1).reshape(
            NWIN_CORE, TOKW, C
        )
    return _window_unpartition(ow)


def run(inputs, trace=False, **kw):
    nc = _get_program()
    in_maps = _make_in_maps(inputs)
    res = run_bass_kernel_spmd(nc, in_maps, list(range(NCORES)), trace=trace, **kw)
    return res


def kernel(**inputs):
    assert int(inputs.get("t_dim", T)) == T
    assert int(inputs.get("h_dim", H)) == H
    assert int(inputs.get("w_dim", W)) == W
    res = run(inputs, trace=False)
    return _assemble(res.results)
